# revision 22
# baseline (speedup 1.0000x reference)
"""Block-diagonal (per-graph) multi-head attention for Trainium2, SPMD over 8 cores.

Problem: nn_MultiHeadAttention (sparse_attention). N=6144 nodes in B=16 graphs
of 384 nodes each (batch ids are sorted and uniform), d_model=256, 8 heads of 32.
The attention mask is block-diagonal, so softmax/attention never crosses graphs.

Sharding: data-parallel over graphs — 2 graphs (768 nodes) per core, the four
256x256 projections replicated.  Each core computes, for its 2 graphs:
  Q^T,K^T = (x W + b)^T in [d, n] layout, V in [n, d] layout (+ones column),
  per (graph, head): S^T = K_h^T(stationary) . Q_h^T -> exp -> attn@V via
  V_aug so one extra product row is the softmax denominator. The unnormalized
  exp-scores ship to the host (the attention-probabilities output is
  block-diagonal; the host divides by the shipped row sums during assembly).
  The context rows are normalized on-device (per-head reciprocal broadcast via
  a tiny selector matmul) and pushed through W_o; b_o is added on the host.

Heads are processed in pairs with explicit PE tile positions: the two heads'
score matmuls (K=32) go to different PE row groups and their attn@V matmuls
(M=33) to different column groups, so each pair streams concurrently.
"""

import os
import sys

for _p in ("/opt/trn_rl_repo", "/root/.axon_site/_ro/trn_rl_repo"):
    if os.path.isdir(_p) and _p not in sys.path:
        sys.path.insert(0, _p)

import numpy as np

import concourse.bass as bass  # noqa: E402
import concourse.tile as tile  # noqa: E402
from concourse import bacc, mybir  # noqa: E402
from concourse.bass_utils import run_bass_kernel_spmd  # noqa: E402
from concourse.masks import make_identity  # noqa: E402

F32 = mybir.dt.float32
F32R = mybir.dt.float32r

N_CORES = 8
B = 16          # graphs
D = 256         # d_model
H = 8           # heads
DK = 32         # head dim
S = 384         # nodes per graph
G = 2           # graphs per core
NL = G * S      # nodes per core = 768
C = S // 128    # 128-chunks per graph = 3
NCH = NL // 128  # node chunks per core = 6
FC = D // 128   # feature chunks = 2
W33 = DK + 1    # head block width in V_aug (ones column appended)
SCALE = 1.0 / float(np.sqrt(DK))

# Flip to False to run all matmuls in full-precision fp32 (4x slower on PE).
USE_F32R = True
MMDT = F32R if USE_F32R else F32


def _build_module():
    nc = bacc.Bacc("TRN2", target_bir_lowering=False, debug=False,
                   num_devices=N_CORES)

    xqt_d = nc.dram_tensor("xqt", [D, NL], MMDT, kind="ExternalInput")
    xkvt_d = nc.dram_tensor("xkvt", [D, NL], MMDT, kind="ExternalInput")
    wall_d = nc.dram_tensor("wall", [4 * D, D], F32, kind="ExternalInput")
    ball_d = nc.dram_tensor("ball", [3, D], F32, kind="ExternalInput")
    sel_d = nc.dram_tensor("selc", [128, 3, 128], MMDT, kind="ExternalInput")

    out_d = nc.dram_tensor("outp", [NL, D], F32, kind="ExternalOutput")
    exps_d = nc.dram_tensor("exps", [G, H, C, 128, S], MMDT, kind="ExternalOutput")
    rsum_d = nc.dram_tensor("rowsum", [FC, G, 4, S], F32, kind="ExternalOutput")

    with tile.TileContext(nc) as tc:
        with (
            nc.allow_low_precision(reason="fp32r matmul operand rounding"),
            tc.tile_pool(name="sing", bufs=1) as sing,
            tc.tile_pool(name="expp", bufs=4) as expp,
            tc.tile_pool(name="pssc", bufs=2, space="PSUM") as pssc,
            tc.tile_pool(name="pssm", bufs=2, space="PSUM") as pssm,
        ):
            # ---- loads ----
            # weights arrive as one stacked tensor, rounded to the matmul dtype
            # in a single cast (walrus requires fp32r matmul operands pre-rounded).
            U16 = mybir.dt.uint16
            w_f = sing.tile([128, 4 * FC, D], F32)
            nc.scalar.dma_start(out=w_f,
                              in_=wall_d[:].rearrange("(w c p) f -> p (w c) f", p=128, c=FC))
            w_all = sing.tile([128, 4 * FC, D], MMDT)
            nc.vector.tensor_copy(w_all, w_f)
            w_sb = {k: w_all[:, i * FC:(i + 1) * FC, :]
                    for i, k in enumerate(("wq", "wk", "wv", "wo"))}
            # bq/bk as per-partition columns [d%128, (b, d//128)] for scalar adds
            bcol = sing.tile([128, 2, FC], F32)
            nc.scalar.dma_start(out=bcol,
                              in_=ball_d[0:2, :].rearrange("b (c p) -> p b c", p=128))
            # bv broadcast across partitions (0-stride partition DMA)
            bv_bc = sing.tile([128, D], F32)
            nc.scalar.dma_start(out=bv_bc, in_=ball_d[2:3, :].to_broadcast((128, D)))
            sel_sb = sing.tile([128, 3, 128], MMDT)
            nc.scalar.dma_start(out=sel_sb, in_=sel_d[:])

            ones_col_f = sing.tile([128, H], F32)
            nc.vector.memset(ones_col_f, 1.0)

            # ---- x arrives pre-transposed from the host ([feature, node]) ----
            xqT = sing.tile([128, FC, NL], MMDT)
            xkvT = sing.tile([128, FC, NL], MMDT)
            nc.sync.dma_start(out=xqT, in_=xqt_d[:].rearrange("(c p) n -> p c n", p=128))
            nc.sync.dma_start(out=xkvT, in_=xkvt_d[:].rearrange("(c p) n -> p c n", p=128))

            # ---- Q^T / K^T projections ([d, n] layout); bias folded into the
            # PSUM->SBUF eviction as a per-partition scalar add ----
            qT = sing.tile([128, FC, NL], MMDT)
            kT = sing.tile([128, FC, NL], MMDT)
            for dst, w, bi, xT in ((qT, w_sb["wq"], 0, xqT),
                                   (kT, w_sb["wk"], 1, xkvT)):
                for fc in range(FC):
                    for nb in range(G):
                        pp = pssm.tile([128, S], F32, tag="ps", name="pp")
                        for kc in range(FC):
                            nc.tensor.matmul(pp, w[:, kc, fc * 128:(fc + 1) * 128],
                                             xT[:, kc, nb * S:(nb + 1) * S],
                                             start=(kc == 0), stop=(kc == FC - 1))
                        nc.vector.tensor_scalar_add(dst[:, fc, nb * S:(nb + 1) * S],
                                                    pp, bcol[:, bi, fc:fc + 1])

            # ---- V in [node, feature] layout, packed per head with a ones col;
            # bv added via a partition-broadcast tile during eviction ----
            vaug = sing.tile([128, NCH, H * W33], MMDT)
            for c in range(NCH):
                nc.vector.tensor_copy(
                    vaug[:, c, :].rearrange("p (h w) -> p h w", w=W33)[:, :, DK:DK + 1],
                    ones_col_f.unsqueeze(2))
                vp = pssm.tile([128, D], F32, tag="ps", name="vp")
                for kc in range(FC):
                    nc.tensor.matmul(vp, xkvT[:, kc, c * 128:(c + 1) * 128],
                                     w_sb["wv"][:, kc, :],
                                     start=(kc == 0), stop=(kc == FC - 1))
                nc.vector.tensor_add(
                    vaug[:, c, :].rearrange("p (h w) -> p h w", w=W33)[:, :, 0:DK],
                    vp.rearrange("p (h w) -> p h w", w=DK),
                    bv_bc.rearrange("p (h w) -> p h w", w=DK))

            # ---- per (graph, head-pair) attention, with per-graph epilogue ----
            aT = sing.tile([128, FC, NL], F32)      # unnormalized context, [d, n]
            # Softmax denominators land on PSUM partition 32; engine APs can
            # only start at partition 0/32/64/96, so stage them on partition 32
            # and scatter with SBUF->SBUF DMA into a lane-parallel layout:
            # rsP partition fc*64 + g*32 + (h%4)*3 + c, free = node%128.
            rsS = sing.tile([DK + 1, H * NL], F32)
            rsP = sing.tile([128, 128], F32)
            nc.vector.memset(rsP, 1.0)              # pad rows: recip stays finite
            rrP = sing.tile([128, 128], MMDT)
            anorm = sing.tile([128, FC, NL], MMDT)
            out_sb = sing.tile([128, NCH, D], F32)
            for g in range(G):
                for hp in range(4):
                    h0, h1 = 2 * hp, 2 * hp + 1
                    fc = hp // 2
                    po0, po1 = (h0 % 4) * DK, (h1 % 4) * DK
                    sc0 = pssc.tile([128, C, 512], F32, tag="sc", name="sc0")
                    sc1 = pssc.tile([128, C, 512], F32, tag="sc", name="sc1")
                    for c in range(C):
                        ksl = slice(g * S + c * 128, g * S + (c + 1) * 128)
                        qsl = slice(g * S, (g + 1) * S)
                        nc.tensor.matmul(sc0[:, c, 0:S], kT[po0:po0 + DK, fc, ksl],
                                         qT[po0:po0 + DK, fc, qsl],
                                         start=True, stop=True, tile_position=(po0, 0))
                        nc.tensor.matmul(sc1[:, c, 0:S], kT[po1:po1 + DK, fc, ksl],
                                         qT[po1:po1 + DK, fc, qsl],
                                         start=True, stop=True, tile_position=(po1, 0))
                    exp2 = expp.tile([128, 2, C, S], MMDT, tag="ex", name="exp2")
                    ex0, ex1 = exp2[:, 0], exp2[:, 1]
                    nc.scalar.activation(ex0, sc0[:, :, 0:S],
                                         mybir.ActivationFunctionType.Exp, scale=SCALE)
                    nc.scalar.activation(ex1, sc1[:, :, 0:S],
                                         mybir.ActivationFunctionType.Exp, scale=SCALE)
                    nc.sync.dma_start(
                        out=exps_d[g, h0:h0 + 2].rearrange("h c p q -> p h c q"),
                        in_=exp2)
                    av0 = pssm.tile([W33, S], F32, tag="ps", name="av0")
                    av1 = pssm.tile([W33, S], F32, tag="ps", name="av1")
                    for c in range(C):
                        nc.tensor.matmul(av0,
                                         vaug[:, g * C + c, h0 * W33:(h0 + 1) * W33],
                                         ex0[:, c, :], start=(c == 0), stop=(c == C - 1))
                        nc.tensor.matmul(av1,
                                         vaug[:, g * C + c, h1 * W33:(h1 + 1) * W33],
                                         ex1[:, c, :], start=(c == 0), stop=(c == C - 1))
                    for h, av in ((h0, av0), (h1, av1)):
                        po = (h % 4) * DK
                        nc.vector.tensor_copy(aT[po:po + DK, fc, g * S:(g + 1) * S],
                                              av[0:DK, :])
                        col = ((fc * 2 + g) * 4 + (h % 4)) * S
                        nc.vector.tensor_copy(rsS[DK:DK + 1, col:col + S],
                                              av[DK:DK + 1, :])

                # ---- per-graph epilogue (overlaps the other graph's attention) ----
                rs_all = rsS[DK:DK + 1, :].rearrange(
                    "o (fc gg j q) -> o fc gg j q", fc=FC, gg=G, j=4)
                nc.scalar.dma_start(out=rsum_d[:, g], in_=rs_all[:, :, g, :, :])
                for fc in range(FC):
                    base = fc * 64 + g * 32
                    bi = (fc * 2 + g) * 4 * S
                    nc.scalar.dma_start(
                        out=rsP[base:base + 12, :],
                        in_=rsS[DK:DK + 1, bi:bi + 4 * S].rearrange(
                            "o (jc p) -> o jc p", p=128))
                    nc.vector.reciprocal(rrP[base:base + 32, :], rsP[base:base + 32, :])
                    rp = pssm.tile([128, S], F32, tag="ps", name="rp")
                    for c in range(C):
                        nc.tensor.matmul(rp[:, c * 128:(c + 1) * 128],
                                         sel_sb[base:base + 12, c, :],
                                         rrP[base:base + 12, :],
                                         start=True, stop=True,
                                         tile_position=(base, 0))
                    nc.vector.tensor_mul(anorm[:, fc, g * S:(g + 1) * S],
                                         aT[:, fc, g * S:(g + 1) * S], rp)
                for lc in range(C):
                    cc = g * C + lc
                    op = pssm.tile([128, D], F32, tag="ps", name="op")
                    for kc in range(FC):
                        nc.tensor.matmul(op, anorm[:, kc, cc * 128:(cc + 1) * 128],
                                         w_sb["wo"][:, kc, :],
                                         start=(kc == 0), stop=(kc == FC - 1))
                    nc.vector.tensor_copy(out_sb[:, cc, :], op)
                nc.scalar.dma_start(
                    out=out_d[:].rearrange("(c p) f -> p c f", p=128)[:, g * C:(g + 1) * C, :],
                    in_=out_sb[:, g * C:(g + 1) * C, :])

    nc.compile()
    return nc


_NC = None


def _get_nc():
    global _NC
    if _NC is None:
        _NC = _build_module()
    return _NC


_SELC = np.zeros((32, 3, 128), np.float32)
for _c in range(3):
    for _p in range(128):
        _SELC[(_p // DK) * 3 + _c, _c, _p] = 1.0
_SELC = np.ascontiguousarray(np.tile(_SELC, (4, 1, 1)))


def _numpy_fallback(x_q, x_kv, Wq, bq, Wk, bk, Wv, bv, Wo, bo, batch_q, batch_kv):
    """Plain numpy reference path for input patterns the device kernel doesn't
    cover (non-uniform graph sizes). Mirrors the reference math in fp32."""
    n_q = x_q.shape[0]
    q = (x_q @ Wq + bq).reshape(n_q, H, DK).transpose(1, 0, 2)
    k = (x_kv @ Wk + bk).reshape(-1, H, DK).transpose(1, 0, 2)
    v = (x_kv @ Wv + bv).reshape(-1, H, DK).transpose(1, 0, 2)
    mask = batch_q[:, None] == batch_kv[None, :]
    scores = np.einsum("hqd,hkd->hqk", q, k).astype(np.float32) * np.float32(SCALE)
    scores = np.where(mask[None], scores, np.float32(-1e30))
    scores -= scores.max(axis=-1, keepdims=True)
    e = np.exp(scores, dtype=np.float32)
    attn = e / e.sum(axis=-1, keepdims=True)
    out = np.einsum("hqk,hkd->hqd", attn, v).transpose(1, 0, 2).reshape(n_q, D)
    out = (out @ Wo + bo).astype(np.float32)
    return out, attn[None].astype(np.float32)


def _run_device(in_maps, trace=False, tmpdir=None):
    nc = _get_nc()
    return run_bass_kernel_spmd(nc, in_maps, list(range(N_CORES)),
                                trace=trace, tmpdir=tmpdir)


def kernel(x_q, x_kv, Wq, bq, Wk, bk, Wv, bv, Wo, bo, batch_q, batch_kv,
           _trace=False, _tmpdir=None, _return_raw=False):
    x_q = np.ascontiguousarray(np.asarray(x_q, np.float32))
    x_kv = np.ascontiguousarray(np.asarray(x_kv, np.float32))
    Wq, Wk, Wv, Wo = (np.ascontiguousarray(np.asarray(w, np.float32))
                      for w in (Wq, Wk, Wv, Wo))
    bq, bk, bv, bo = (np.ascontiguousarray(np.asarray(b, np.float32).reshape(1, D))
                      for b in (bq, bk, bv, bo))
    batch_q = np.asarray(batch_q)
    batch_kv = np.asarray(batch_kv)

    pattern = np.repeat(np.arange(B), S)
    if (x_q.shape != (B * S, D) or x_kv.shape != (B * S, D)
            or not np.array_equal(batch_q, pattern)
            or not np.array_equal(batch_kv, pattern)):
        return _numpy_fallback(x_q, x_kv, Wq, bq[0], Wk, bk[0], Wv, bv[0],
                               Wo, bo[0], batch_q, batch_kv)

    wall = np.ascontiguousarray(np.concatenate([Wq, Wk, Wv, Wo], axis=0))
    ball = np.ascontiguousarray(np.concatenate([bq, bk, bv], axis=0))
    in_maps = []
    for i in range(N_CORES):
        rows = slice(i * NL, (i + 1) * NL)
        in_maps.append({
            "xqt": np.ascontiguousarray(x_q[rows].T),
            "xkvt": np.ascontiguousarray(x_kv[rows].T),
            "wall": wall, "ball": ball,
            "selc": _SELC,
        })
    res = _run_device(in_maps, trace=_trace, tmpdir=_tmpdir)

    out = np.concatenate([res.results[i]["outp"] for i in range(N_CORES)], axis=0)
    out += bo  # b_o is a per-feature add after the output projection
    attn = np.zeros((1, H, B * S, B * S), np.float32)
    for i in range(N_CORES):
        exps = res.results[i]["exps"]          # [G, H, C, 128, S]  (k-major, q-minor)
        rsum = res.results[i]["rowsum"]        # [FC, G, 4, S]
        for g in range(G):
            gg = i * G + g
            blk = exps[g].reshape(H, S, S)     # [h, k, q]
            rs = rsum[:, g].reshape(H, S)      # [h, q]
            sl = slice(gg * S, (gg + 1) * S)
            attn[0, :, sl, sl] = (blk / rs[:, None, :]).transpose(0, 2, 1)
    if _return_raw:
        return (out, attn), res
    return out, attn


# revision 23
# speedup vs baseline: 1.1021x; 1.1021x over previous
"""Block-diagonal (per-graph) multi-head attention for Trainium2, SPMD over 8 cores.

Problem: nn_MultiHeadAttention (sparse_attention). N=6144 nodes in B=16 graphs
of 384 nodes each (batch ids are sorted and uniform), d_model=256, 8 heads of 32.
The attention mask is block-diagonal, so softmax/attention never crosses graphs.

Sharding: data-parallel over graphs — 2 graphs (768 nodes) per core, the four
256x256 projections replicated.  Each core computes, for its 2 graphs:
  Q^T,K^T = (x W + b)^T in [d, n] layout, V in [n, d] layout (+ones column),
  per (graph, head): S^T = K_h^T(stationary) . Q_h^T -> exp -> attn@V via
  V_aug so one extra product row is the softmax denominator. The unnormalized
  exp-scores ship to the host (the attention-probabilities output is
  block-diagonal; the host divides by the shipped row sums during assembly).
  The context rows are normalized on-device (per-head reciprocal broadcast via
  a tiny selector matmul) and pushed through W_o; b_o is added on the host.

Heads are processed in pairs with explicit PE tile positions: the two heads'
score matmuls (K=32) go to different PE row groups and their attn@V matmuls
(M=33) to different column groups, so each pair streams concurrently.
"""

import os
import sys

for _p in ("/opt/trn_rl_repo", "/root/.axon_site/_ro/trn_rl_repo"):
    if os.path.isdir(_p) and _p not in sys.path:
        sys.path.insert(0, _p)

import numpy as np

import concourse.bass as bass  # noqa: E402
import concourse.tile as tile  # noqa: E402
from concourse import bacc, mybir  # noqa: E402
from concourse.bass_utils import run_bass_kernel_spmd  # noqa: E402
from concourse.masks import make_identity  # noqa: E402

F32 = mybir.dt.float32
F32R = mybir.dt.float32r

N_CORES = 8
B = 16          # graphs
D = 256         # d_model
H = 8           # heads
DK = 32         # head dim
S = 384         # nodes per graph
G = 2           # graphs per core
NL = G * S      # nodes per core = 768
C = S // 128    # 128-chunks per graph = 3
NCH = NL // 128  # node chunks per core = 6
FC = D // 128   # feature chunks = 2
W33 = DK + 1    # head block width in V_aug (ones column appended)
SCALE = 1.0 / float(np.sqrt(DK))

# Flip to False to run all matmuls in full-precision fp32 (4x slower on PE).
USE_F32R = True
MMDT = F32R if USE_F32R else F32


def _build_module():
    nc = bacc.Bacc("TRN2", target_bir_lowering=False, debug=False,
                   num_devices=N_CORES)

    xqt_d = nc.dram_tensor("xqt", [D, NL], MMDT, kind="ExternalInput")
    xkvt_d = nc.dram_tensor("xkvt", [D, NL], MMDT, kind="ExternalInput")
    wall_d = nc.dram_tensor("wall", [4 * D, D], F32, kind="ExternalInput")
    ball_d = nc.dram_tensor("ball", [3, D], F32, kind="ExternalInput")
    sel_d = nc.dram_tensor("selc", [128, 3, 128], MMDT, kind="ExternalInput")

    out_d = nc.dram_tensor("outp", [NL, D], F32, kind="ExternalOutput")
    exps_d = nc.dram_tensor("exps", [G, H, C, 128, S], MMDT, kind="ExternalOutput")
    rsum_d = nc.dram_tensor("rowsum", [FC, G, 4, S], F32, kind="ExternalOutput")

    with tile.TileContext(nc) as tc:
        with (
            nc.allow_low_precision(reason="fp32r matmul operand rounding"),
            tc.tile_pool(name="sing", bufs=1) as sing,
            tc.tile_pool(name="expp", bufs=4) as expp,
            tc.tile_pool(name="pssc", bufs=2, space="PSUM") as pssc,
            tc.tile_pool(name="pssm", bufs=2, space="PSUM") as pssm,
        ):
            # ---- loads ----
            # weights arrive as one stacked tensor, rounded to the matmul dtype
            # in a single cast (walrus requires fp32r matmul operands pre-rounded).
            w_f = sing.tile([128, 4 * FC, D], F32)
            nc.sync.dma_start(out=w_f,
                              in_=wall_d[:].rearrange("(w c p) f -> p (w c) f", p=128, c=FC))
            w_all = sing.tile([128, 4 * FC, D], MMDT)
            for wi in range(4):
                nc.vector.tensor_copy(w_all[:, wi * FC:(wi + 1) * FC, :],
                                      w_f[:, wi * FC:(wi + 1) * FC, :])
            w_sb = {k: w_all[:, i * FC:(i + 1) * FC, :]
                    for i, k in enumerate(("wq", "wk", "wv", "wo"))}
            # ---- x arrives pre-transposed from the host ([feature, node]) ----
            xqT = sing.tile([128, FC, NL], MMDT)
            xkvT = sing.tile([128, FC, NL], MMDT)
            nc.sync.dma_start(out=xqT, in_=xqt_d[:].rearrange("(c p) n -> p c n", p=128))
            nc.sync.dma_start(out=xkvT, in_=xkvt_d[:].rearrange("(c p) n -> p c n", p=128))
            # small/broadcast loads on the otherwise-idle SWDGE queue
            # bq/bk as per-partition columns [d%128, (b, d//128)] for scalar adds
            bcol = sing.tile([128, 2, FC], F32)
            nc.gpsimd.dma_start(out=bcol,
                                in_=ball_d[0:2, :].rearrange("b (c p) -> p b c", p=128))
            # bv broadcast across partitions (0-stride partition DMA)
            bv_bc = sing.tile([128, D], F32)
            nc.gpsimd.dma_start(out=bv_bc, in_=ball_d[2:3, :].to_broadcast((128, D)))
            sel_sb = sing.tile([128, 3, 128], MMDT)
            nc.gpsimd.dma_start(out=sel_sb, in_=sel_d[:])

            ones_col_f = sing.tile([128, H], F32)
            nc.vector.memset(ones_col_f, 1.0)

            # ---- Q^T / K^T projections ([d, n] layout); bias folded into the
            # PSUM->SBUF eviction as a per-partition scalar add ----
            qT = sing.tile([128, FC, NL], MMDT)
            kT = sing.tile([128, FC, NL], MMDT)
            for dst, w, bi, xT in ((qT, w_sb["wq"], 0, xqT),
                                   (kT, w_sb["wk"], 1, xkvT)):
                for fc in range(FC):
                    for nb in range(G):
                        pp = pssm.tile([128, S], F32, tag="ps", name="pp")
                        for kc in range(FC):
                            nc.tensor.matmul(pp, w[:, kc, fc * 128:(fc + 1) * 128],
                                             xT[:, kc, nb * S:(nb + 1) * S],
                                             start=(kc == 0), stop=(kc == FC - 1))
                        nc.vector.tensor_scalar_add(dst[:, fc, nb * S:(nb + 1) * S],
                                                    pp, bcol[:, bi, fc:fc + 1])

            # ---- V in [node, feature] layout, packed per head with a ones col;
            # bv added via a partition-broadcast tile during eviction ----
            vaug = sing.tile([128, NCH, H * W33], MMDT)
            for c in range(NCH):
                nc.vector.tensor_copy(
                    vaug[:, c, :].rearrange("p (h w) -> p h w", w=W33)[:, :, DK:DK + 1],
                    ones_col_f.unsqueeze(2))
                vp = pssm.tile([128, D], F32, tag="ps", name="vp")
                for kc in range(FC):
                    nc.tensor.matmul(vp, xkvT[:, kc, c * 128:(c + 1) * 128],
                                     w_sb["wv"][:, kc, :],
                                     start=(kc == 0), stop=(kc == FC - 1))
                nc.vector.tensor_add(
                    vaug[:, c, :].rearrange("p (h w) -> p h w", w=W33)[:, :, 0:DK],
                    vp.rearrange("p (h w) -> p h w", w=DK),
                    bv_bc.rearrange("p (h w) -> p h w", w=DK))

            # ---- per (graph, head-pair) attention, with per-graph epilogue ----
            aT = sing.tile([128, FC, NL], F32)      # unnormalized context, [d, n]
            # Softmax denominators land on PSUM partition 32; engine APs can
            # only start at partition 0/32/64/96, so stage them on partition 32
            # and scatter with SBUF->SBUF DMA into a lane-parallel layout:
            # rsP partition fc*64 + g*32 + (h%4)*3 + c, free = node%128.
            rsS = sing.tile([DK + 1, H * NL], F32)
            rsP = sing.tile([128, 128], F32)
            nc.vector.memset(rsP, 1.0)              # pad rows: recip stays finite
            rrP = sing.tile([128, 128], MMDT)
            anorm = sing.tile([128, FC, NL], MMDT)
            out_sb = sing.tile([128, NCH, D], F32)
            for g in range(G):
                for hp in range(4):
                    h0, h1 = 2 * hp, 2 * hp + 1
                    fc = hp // 2
                    po0, po1 = (h0 % 4) * DK, (h1 % 4) * DK
                    sc0 = pssc.tile([128, C, 512], F32, tag="sc", name="sc0")
                    sc1 = pssc.tile([128, C, 512], F32, tag="sc", name="sc1")
                    for c in range(C):
                        ksl = slice(g * S + c * 128, g * S + (c + 1) * 128)
                        qsl = slice(g * S, (g + 1) * S)
                        nc.tensor.matmul(sc0[:, c, 0:S], kT[po0:po0 + DK, fc, ksl],
                                         qT[po0:po0 + DK, fc, qsl],
                                         start=True, stop=True, tile_position=(po0, 0))
                        nc.tensor.matmul(sc1[:, c, 0:S], kT[po1:po1 + DK, fc, ksl],
                                         qT[po1:po1 + DK, fc, qsl],
                                         start=True, stop=True, tile_position=(po1, 0))
                    exp2 = expp.tile([128, 2, C, S], MMDT, tag="ex", name="exp2")
                    ex0, ex1 = exp2[:, 0], exp2[:, 1]
                    nc.scalar.activation(ex0, sc0[:, :, 0:S],
                                         mybir.ActivationFunctionType.Exp, scale=SCALE)
                    nc.scalar.activation(ex1, sc1[:, :, 0:S],
                                         mybir.ActivationFunctionType.Exp, scale=SCALE)
                    nc.sync.dma_start(
                        out=exps_d[g, h0:h0 + 2].rearrange("h c p q -> p h c q"),
                        in_=exp2)
                    av0 = pssm.tile([W33, S], F32, tag="ps", name="av0")
                    av1 = pssm.tile([W33, S], F32, tag="ps", name="av1")
                    for c in range(C):
                        nc.tensor.matmul(av0,
                                         vaug[:, g * C + c, h0 * W33:(h0 + 1) * W33],
                                         ex0[:, c, :], start=(c == 0), stop=(c == C - 1))
                        nc.tensor.matmul(av1,
                                         vaug[:, g * C + c, h1 * W33:(h1 + 1) * W33],
                                         ex1[:, c, :], start=(c == 0), stop=(c == C - 1))
                    for h, av in ((h0, av0), (h1, av1)):
                        po = (h % 4) * DK
                        nc.vector.tensor_copy(aT[po:po + DK, fc, g * S:(g + 1) * S],
                                              av[0:DK, :])
                        col = ((fc * 2 + g) * 4 + (h % 4)) * S
                        nc.vector.tensor_copy(rsS[DK:DK + 1, col:col + S],
                                              av[DK:DK + 1, :])

                # ---- per-graph epilogue (overlaps the other graph's attention) ----
                rs_all = rsS[DK:DK + 1, :].rearrange(
                    "o (fc gg j q) -> o fc gg j q", fc=FC, gg=G, j=4)
                nc.scalar.dma_start(out=rsum_d[:, g], in_=rs_all[:, :, g, :, :])
                for fc in range(FC):
                    base = fc * 64 + g * 32
                    bi = (fc * 2 + g) * 4 * S
                    nc.scalar.dma_start(
                        out=rsP[base:base + 12, :],
                        in_=rsS[DK:DK + 1, bi:bi + 4 * S].rearrange(
                            "o (jc p) -> o jc p", p=128))
                    nc.vector.reciprocal(rrP[base:base + 32, :], rsP[base:base + 32, :])
                    rp = pssm.tile([128, S], F32, tag="ps", name="rp")
                    for c in range(C):
                        nc.tensor.matmul(rp[:, c * 128:(c + 1) * 128],
                                         sel_sb[base:base + 12, c, :],
                                         rrP[base:base + 12, :],
                                         start=True, stop=True,
                                         tile_position=(base, 0))
                    nc.vector.tensor_mul(anorm[:, fc, g * S:(g + 1) * S],
                                         aT[:, fc, g * S:(g + 1) * S], rp)
                for lc in range(C):
                    cc = g * C + lc
                    op = pssm.tile([128, D], F32, tag="ps", name="op")
                    for kc in range(FC):
                        nc.tensor.matmul(op, anorm[:, kc, cc * 128:(cc + 1) * 128],
                                         w_sb["wo"][:, kc, :],
                                         start=(kc == 0), stop=(kc == FC - 1))
                    nc.vector.tensor_copy(out_sb[:, cc, :], op)
                nc.scalar.dma_start(
                    out=out_d[:].rearrange("(c p) f -> p c f", p=128)[:, g * C:(g + 1) * C, :],
                    in_=out_sb[:, g * C:(g + 1) * C, :])

    nc.compile()
    return nc


_NC = None


def _get_nc():
    global _NC
    if _NC is None:
        _NC = _build_module()
    return _NC


_SELC = np.zeros((32, 3, 128), np.float32)
for _c in range(3):
    for _p in range(128):
        _SELC[(_p // DK) * 3 + _c, _c, _p] = 1.0
_SELC = np.ascontiguousarray(np.tile(_SELC, (4, 1, 1)))


def _numpy_fallback(x_q, x_kv, Wq, bq, Wk, bk, Wv, bv, Wo, bo, batch_q, batch_kv):
    """Plain numpy reference path for input patterns the device kernel doesn't
    cover (non-uniform graph sizes). Mirrors the reference math in fp32."""
    n_q = x_q.shape[0]
    q = (x_q @ Wq + bq).reshape(n_q, H, DK).transpose(1, 0, 2)
    k = (x_kv @ Wk + bk).reshape(-1, H, DK).transpose(1, 0, 2)
    v = (x_kv @ Wv + bv).reshape(-1, H, DK).transpose(1, 0, 2)
    mask = batch_q[:, None] == batch_kv[None, :]
    scores = np.einsum("hqd,hkd->hqk", q, k).astype(np.float32) * np.float32(SCALE)
    scores = np.where(mask[None], scores, np.float32(-1e30))
    scores -= scores.max(axis=-1, keepdims=True)
    e = np.exp(scores, dtype=np.float32)
    attn = e / e.sum(axis=-1, keepdims=True)
    out = np.einsum("hqk,hkd->hqd", attn, v).transpose(1, 0, 2).reshape(n_q, D)
    out = (out @ Wo + bo).astype(np.float32)
    return out, attn[None].astype(np.float32)


def _run_device(in_maps, trace=False, tmpdir=None):
    nc = _get_nc()
    return run_bass_kernel_spmd(nc, in_maps, list(range(N_CORES)),
                                trace=trace, tmpdir=tmpdir)


def kernel(x_q, x_kv, Wq, bq, Wk, bk, Wv, bv, Wo, bo, batch_q, batch_kv,
           _trace=False, _tmpdir=None, _return_raw=False):
    x_q = np.ascontiguousarray(np.asarray(x_q, np.float32))
    x_kv = np.ascontiguousarray(np.asarray(x_kv, np.float32))
    Wq, Wk, Wv, Wo = (np.ascontiguousarray(np.asarray(w, np.float32))
                      for w in (Wq, Wk, Wv, Wo))
    bq, bk, bv, bo = (np.ascontiguousarray(np.asarray(b, np.float32).reshape(1, D))
                      for b in (bq, bk, bv, bo))
    batch_q = np.asarray(batch_q)
    batch_kv = np.asarray(batch_kv)

    pattern = np.repeat(np.arange(B), S)
    if (x_q.shape != (B * S, D) or x_kv.shape != (B * S, D)
            or not np.array_equal(batch_q, pattern)
            or not np.array_equal(batch_kv, pattern)):
        return _numpy_fallback(x_q, x_kv, Wq, bq[0], Wk, bk[0], Wv, bv[0],
                               Wo, bo[0], batch_q, batch_kv)

    wall = np.ascontiguousarray(np.concatenate([Wq, Wk, Wv, Wo], axis=0))
    ball = np.ascontiguousarray(np.concatenate([bq, bk, bv], axis=0))
    in_maps = []
    for i in range(N_CORES):
        rows = slice(i * NL, (i + 1) * NL)
        in_maps.append({
            "xqt": np.ascontiguousarray(x_q[rows].T),
            "xkvt": np.ascontiguousarray(x_kv[rows].T),
            "wall": wall, "ball": ball,
            "selc": _SELC,
        })
    res = _run_device(in_maps, trace=_trace, tmpdir=_tmpdir)

    out = np.concatenate([res.results[i]["outp"] for i in range(N_CORES)], axis=0)
    out += bo  # b_o is a per-feature add after the output projection
    attn = np.zeros((1, H, B * S, B * S), np.float32)
    for i in range(N_CORES):
        exps = res.results[i]["exps"]          # [G, H, C, 128, S]  (k-major, q-minor)
        rsum = res.results[i]["rowsum"]        # [FC, G, 4, S]
        for g in range(G):
            gg = i * G + g
            blk = exps[g].reshape(H, S, S)     # [h, k, q]
            rs = rsum[:, g].reshape(H, S)      # [h, q]
            sl = slice(gg * S, (gg + 1) * S)
            attn[0, :, sl, sl] = (blk / rs[:, None, :]).transpose(0, 2, 1)
    if _return_raw:
        return (out, attn), res
    return out, attn
